# revision 42
# baseline (speedup 1.0000x reference)
"""Causal self-attention kernel for 8 Trainium2 NeuronCores.

Sharding: core c -> (batch b = c//2, head-group g = c%2). Each core computes
the attention output contribution of 8 heads for one batch element:
    P_c = (sum_{h in group} softmax(Q_h K_h^T / 8 + causal) V_h) @ WO
Host epilogue: out[b] = P_{2b} + P_{2b+1} + (sum_h bV_h) @ WO + 16*bO
(the V-bias commutes through softmax normalization: softmax rows sum to 1;
the K-bias cancels entirely: softmax((Q+bq)(K+bk)^T) = softmax((Q+bq)K^T)
because Q.bk is constant along the key axis.)

v4 design notes (evolved from v3, 339.7us):
  - Projections and scores fp16; ET fp8e4m3; V as fp8 V8 + fp8 residual R8
    consumed by DoubleRow A@V (unchanged math from v3).
  - K projection carries NO bias -> evicted by ScalarE (activation Copy);
    V8 cast f32->fp8 also on ScalarE; both run in proj phases where the
    scalar engine is otherwise idle. DVE keeps Q bias add + R8 residual.
  - Normalization fused into PSUM eviction: copy the l-row to SBUF, PE-
    broadcast it with a ones[1,64] stationary matmul into a PSUM tile,
    reciprocal_approx_fast, then tensor_tensor multiply zt (PSUM) straight
    into zsum.  The v3 ztall intermediate, its 32 DVE copies, and the
    DRAM-bounce partition broadcast are gone.
  - Cross-head-pair software pipelining: the last AV pairs + the norm chain
    of head-pair hp issue as a deferred "carry" inside hp+1's score stream
    (after 3 score tiles), removing the per-hp PE drain bubble.
  - Projection work is sliced into ~1us filler units; attention(qc)
    consumes units of chunk qc+1's projection at hp boundaries, the rest
    issue between chunks. Keeps the PE fed where ScalarE exp lags.
  - x loaded as column-half whole-chunk DMAs split across the sync and
    vector queues, all issued upfront (xs pool holds all 4 chunks).
  - Output is f16 (host upcasts): halves the out DMA; out DMAs ride the
    gpsimd queue which is idle after the weight loads.
"""
import numpy as np

B, S, D, H, DH = 4, 2048, 1024, 16, 64
HPC = 8            # heads per core
GD = HPC * DH      # 512 = group width
NCORES = 8
NQ = S // 512      # 4 q/s chunks of 512
NKT = S // 128     # 16 k-tiles
NDT = D // 128     # 8 d-tiles

_prog = {}


def bass_ap_3d(tile_t, offset, stride, n, inner):
    """AP view [128p, n, inner] over a tile's free dim: col = offset + i*stride + c."""
    import concourse.bass as bass
    ap = tile_t[:]
    return bass.AP(ap.tensor, ap.offset + offset,
                   [ap.ap[0], [stride, n], [1, inner]])


def _build():
    import concourse.bacc as bacc
    import concourse.tile as tile
    from concourse import mybir
    import concourse.bass as bass

    f32 = mybir.dt.float32
    f16 = mybir.dt.float16
    f8 = mybir.dt.float8e4
    AF = mybir.ActivationFunctionType
    ALU = mybir.AluOpType
    DR = mybir.MatmulPerfMode.DoubleRow

    nc = bacc.Bacc(None, target_bir_lowering=False, debug=False)
    # x and the projection weights arrive host-pretiled into the SBUF
    # layouts (contiguous per partition -> cheap DMA descriptors):
    #   x[p, c*4096 + t*1024 + col]  = x_orig[c*512 + t*128 + p, col]
    #   w[p, k*GD + col]             = W_orig[k*128 + p, col]
    x = nc.dram_tensor("x", [128, NQ * 4096], f16, kind="ExternalInput")
    wq = nc.dram_tensor("wq", [128, NDT * GD], f16, kind="ExternalInput")
    wk = nc.dram_tensor("wk", [128, NDT * GD], f16, kind="ExternalInput")
    wv = nc.dram_tensor("wv", [128, NDT * GD], f16, kind="ExternalInput")
    bq = nc.dram_tensor("bq", [1, GD], f16, kind="ExternalInput")
    wo = nc.dram_tensor("wo", [DH, D], f16, kind="ExternalInput")
    idtd = nc.dram_tensor("idt", [128, 128], f16, kind="ExternalInput")
    out = nc.dram_tensor("out", [S, D], f16, kind="ExternalOutput")

    with tile.TileContext(nc) as tc:
        with tc.tile_pool(name="const", bufs=1) as constp, \
             tc.tile_pool(name="big", bufs=1) as bigp:
            # ---- persistent tensors ----
            xs_all = bigp.tile([128, NQ * 4096], f16, tag="xs")   # chunk c at c*4096
            xt_all = bigp.tile([128, NDT * S], f16, tag="xt")     # d-tile j at j*S
            qt_all = bigp.tile([128, 4 * S], f16, tag="qt")       # m-tile m at m*S
            kt_all = bigp.tile([128, 4 * S], f16, tag="kt")
            vt_all = bigp.tile([128, NKT * 528], f8, tag="vt")    # ones+V8 cols
            rt_all = bigp.tile([128, NKT * 528], f8, tag="rt")    # fp8 residual
            # rows 1-64 hold sum_h Z_h/l_h (row 0 unused: keeps partition
            # alignment with zt, whose row 0 is the l accumulator); f16 so
            # the tail zsr DMAs don't cast (casting DMAs are gpsimd-only)
            zsum = bigp.tile([DH + 1, S], f16, tag="zsum")

            idt = constp.tile([128, 128], f16, tag="idt")
            bq_t = constp.tile([128, 4], f32, tag="bq_t")
            wo_sb = constp.tile([128, D], f16, tag="wo_sb")
            ones_sb = constp.tile([1, DH + 1], f32, tag="ones_sb")

            # ---- input DMAs: x chunk 0 split into column halves across the
            # sync and scalar queues so the first transposes start ~1.5us in;
            # chunks 1-3 follow the weights on the gpsimd queue (needed much
            # later, keeps HBM clear for the weights) ----
            # x arrives host-pretiled as [128, NQ*4096] (contiguous per
            # partition: cheap DMA descriptors)
            def x_chunk_dma(c, half, eng):
                dst = bass.AP(xs_all[:].tensor,
                              xs_all[:].offset + c * 4096 + half * 2048,
                              [xs_all[:].ap[0], [1, 2048]])
                src = bass.AP(x, c * 4096 + half * 2048,
                              [[NQ * 4096, 128], [1, 2048]])
                eng.dma_start(dst, src)
            # scalar queue: idt first (gates the transposes), then x c0h1
            nc.scalar.dma_start(idt[:], idtd[:])
            x_chunk_dma(0, 1, nc.scalar)
            # bq_t is a casting DMA (f16->f32): gpsimd-only
            nc.gpsimd.dma_start(bq_t[:], bass.AP(bq, 0, [[1, 128], [128, 4]]))
            # sync queue: x c0h0, then chunks 1-3 whole
            x_chunk_dma(0, 0, nc.sync)

            with tc.tile_pool(name="wts", bufs=1) as wtp, \
                 tc.tile_pool(name="et", bufs=6) as etp, \
                 tc.tile_pool(name="lrow", bufs=2) as lrp, \
                 tc.tile_pool(name="rld", bufs=3, space="DRAM") as rldp, \
                 tc.tile_pool(name="lbi", bufs=2) as lbip, \
                 tc.tile_pool(name="zn", bufs=2) as znp, \
                 tc.tile_pool(name="zr", bufs=2) as zrp, \
                 tc.tile_pool(name="osb", bufs=3) as osbp, \
                 tc.tile_pool(name="stp", bufs=2, space="PSUM") as stp, \
                 tc.tile_pool(name="ppp", bufs=2, space="PSUM") as ppp, \
                 tc.tile_pool(name="ztp", bufs=2, space="PSUM") as ztp:
                wq_all = wtp.tile([128, NDT * GD], f16, tag="wq_all")
                wk_all = wtp.tile([128, NDT * GD], f16, tag="wk_all")
                wv_all = wtp.tile([128, NDT * GD], f16, tag="wv_all")
                for (w_all, w_dram) in ((wq_all, wq), (wk_all, wk),
                                        (wv_all, wv)):
                    nc.gpsimd.dma_start(w_all[:], w_dram[:])
                # x chunks 1-3 on the sync queue behind c0h0
                for c in range(1, NQ):
                    x_chunk_dma(c, 0, nc.sync)
                    x_chunk_dma(c, 1, nc.sync)
                # vt/rt layout per (kt, head): col 0 = ones (the l
                # accumulator lands at PSUM partition 0 so the reciprocal
                # can read it directly), cols 1-64 = V8, col 65 = pad.
                nc.gpsimd.memset(ones_sb[:], 1.0)
                nc.gpsimd.memset(
                    bass_ap_3d(vt_all, 0, 66, NKT * HPC, 1), 1.0)
                nc.gpsimd.memset(
                    bass_ap_3d(vt_all, 65, 66, NKT * HPC, 1), 0.0)
                nc.gpsimd.memset(
                    bass.AP(rt_all[:].tensor, rt_all[:].offset,
                            [rt_all[:].ap[0], [66, NKT * HPC], [65, 2]]),
                    0.0)
                nc.gpsimd.dma_start(wo_sb[0:DH, :], wo[:])
                nc.gpsimd.dma_start(wo_sb[DH:2 * DH, :], wo[:])

                # ---------- projection filler units ----------
                def transpose_unit(c, jj):
                    # transposes d-tiles 2jj, 2jj+1 of chunk c
                    pt = ppp.tile([128, 1024], f16, tag="pp", name="pt")
                    for j2 in range(2):
                        j = jj * 2 + j2
                        for st4 in range(4):
                            col = c * 4096 + st4 * 1024 + j * 128
                            nc.tensor.transpose(
                                pt[:, j2 * 512 + st4 * 128:
                                   j2 * 512 + (st4 + 1) * 128],
                                xs_all[:, col:col + 128],
                                idt[:])
                    dst = bass_ap_3d(xt_all, (jj * 2) * S + c * 512, S, 2, 512)
                    nc.vector.tensor_copy(dst, bass_ap_3d(pt, 0, 512, 2, 512))

                def qkproj_unit(c, which, m):
                    # one m-tile (2 heads) of the Q or K projection of chunk c
                    w_all = wq_all if which == 0 else wk_all
                    dest = qt_all if which == 0 else kt_all
                    ps = ppp.tile([128, 512], f32, tag="pp", name="ps")
                    for k in range(NDT):
                        nc.tensor.matmul(
                            ps[:],
                            w_all[:, k * GD + m * 128: k * GD + (m + 1) * 128],
                            xt_all[:, k * S + c * 512: k * S + (c + 1) * 512],
                            start=(k == 0), stop=(k == NDT - 1))
                    dcols = dest[:, m * S + c * 512: m * S + (c + 1) * 512]
                    if which == 0:
                        nc.vector.tensor_scalar_add(dcols, ps[:],
                                                    bq_t[:, m:m + 1])
                    else:
                        nc.vector.tensor_copy(dcols, ps[:])

                def vproj_unit(c, st4):
                    st = c * 4 + st4
                    ps = ppp.tile([128, 512], f32, tag="pp", name="ps")
                    for k in range(NDT):
                        nc.tensor.matmul(
                            ps[:],
                            xt_all[:, k * S + st * 128: k * S + (st + 1) * 128],
                            wv_all[:, k * GD:(k + 1) * GD],
                            start=(k == 0), stop=(k == NDT - 1))
                    dst = bass_ap_3d(vt_all, st * 528 + 1, 66, HPC, DH)
                    srcap = bass_ap_3d(ps, 0, DH, HPC, DH)
                    nc.vector.tensor_copy(dst, srcap)
                    rdst = bass_ap_3d(rt_all, st * 528 + 1, 66, HPC, DH)
                    nc.vector.tensor_tensor(rdst, srcap, dst, op=ALU.subtract)

                def proj_units(c):
                    units = []
                    for jj in range(4):
                        units.append(lambda c=c, jj=jj: transpose_unit(c, jj))
                    for which in range(2):
                        for m in range(4):
                            units.append(lambda c=c, w=which, m=m:
                                         qkproj_unit(c, w, m))
                    for st4 in range(4):
                        units.append(lambda c=c, s=st4: vproj_unit(c, s))
                    return units

                # ---------- attention ----------
                def make_drain(qc, hp, zt0, zt1, pending, npairs,
                               final=False):
                    """Deferred: last AVs of (qc,hp), then fused norm."""
                    def av(pp, pet):
                        jz = max(2 * pp - 4 * qc, 0)
                        q0 = jz * 128
                        for half, zt in ((0, zt0), (1, zt1)):
                            eap = bass_ap_3d(pet, half * 512 + q0,
                                             1024, 2, 512 - q0)
                            voff = (2 * pp) * 528 + (2 * hp + half) * 66
                            nc.tensor.matmul(
                                zt[:, q0:512],
                                bass_ap_3d(vt_all, voff, 528, 2, 66),
                                eap, start=(pp == 0), stop=False,
                                perf_mode=DR)
                            nc.tensor.matmul(
                                zt[:, q0:512],
                                bass_ap_3d(rt_all, voff, 528, 2, 66),
                                eap, start=False,
                                stop=(pp == npairs - 1),
                                perf_mode=DR)

                    def drain():
                        while pending:
                            av(*pending.pop(0))
                        # fused normalization: 1/l straight off PSUM row 0,
                        # gpsimd broadcast, multiply into zsum rows 1-64
                        zcols = slice(qc * 512, (qc + 1) * 512)
                        for half, zt in ((0, zt0), (1, zt1)):
                            lrcp = lrp.tile([1, 512], f32, tag="lrcp")
                            nc.vector.reciprocal_approx_fast(
                                out=lrcp[:], in_=zt[0:1, :])
                            lbi = lbip.tile([DH + 1, 512], f32, tag="lbi")
                            if final:
                                # sync is busy with the tail DMAs at the
                                # end; broadcast on the idle PE + scalar
                                lbp = ppp.tile([DH + 1, 512], f32, tag="pp",
                                               name="lbp")
                                nc.tensor.matmul(lbp[:], ones_sb[:],
                                                 lrcp[:], start=True,
                                                 stop=True)
                                nc.scalar.activation(lbi[:], lbp[:],
                                                     AF.Copy)
                            else:
                                # broadcast via DRAM bounce on the idle
                                # sync queue (keeps gpsimd/PE/scalar out
                                # of the DVE-ordered norm chain)
                                rld = rldp.tile([1, 512], f32, tag="rld")
                                nc.sync.dma_start(rld[:], lrcp[:])
                                rap = rld[:]
                                nc.sync.dma_start(
                                    lbi[:],
                                    bass.AP(rap.tensor, rap.offset,
                                            [[0, DH + 1], [1, 512]]))
                            # DVE PSUM reads must sit at partition base 0:
                            # span rows 0-64 (row 0 computes l*(1/l) into
                            # the unused zsum row 0)
                            if 2 * hp + half == 0:
                                nc.vector.tensor_tensor(
                                    zsum[0:DH + 1, zcols], zt[0:DH + 1, :],
                                    lbi[0:DH + 1, :], op=ALU.mult)
                            else:
                                zn = znp.tile([DH + 1, 512], f16, tag="zn")
                                nc.vector.tensor_tensor(
                                    zn[0:DH + 1, :], zt[0:DH + 1, :],
                                    lbi[0:DH + 1, :], op=ALU.mult)
                                nc.vector.tensor_tensor(
                                    zsum[0:DH + 1, zcols],
                                    zsum[0:DH + 1, zcols],
                                    zn[0:DH + 1, :], op=ALU.add)
                    return drain

                def attention(qc, carry, filler, posts):
                    """carry: deferred drain from the previous (qc,hp);
                    filler: proj units to interleave; posts: deferred
                    tail-projection units of the previous chunk."""
                    ktiles = 4 * qc + 4
                    npairs = ktiles // 2
                    for hp in range(4):
                        zt0 = ztp.tile([66, 512], f32, tag="zt", name="zt0")
                        zt1 = ztp.tile([66, 512], f32, tag="zt", name="zt1")
                        pending = []

                        def av_flush(lag):
                            while len(pending) > lag:
                                pp, pet = pending.pop(0)
                                jz = max(2 * pp - 4 * qc, 0)
                                q0 = jz * 128
                                for half, zt in ((0, zt0), (1, zt1)):
                                    eap = bass_ap_3d(pet, half * 512 + q0,
                                                     1024, 2, 512 - q0)
                                    voff = ((2 * pp) * 528
                                            + (2 * hp + half) * 66)
                                    nc.tensor.matmul(
                                        zt[:, q0:512],
                                        bass_ap_3d(vt_all, voff, 528, 2, 66),
                                        eap, start=(pp == 0), stop=False,
                                        perf_mode=DR)
                                    nc.tensor.matmul(
                                        zt[:, q0:512],
                                        bass_ap_3d(rt_all, voff, 528, 2, 66),
                                        eap, start=False,
                                        stop=(pp == npairs - 1),
                                        perf_mode=DR)

                        et2 = None
                        for kt in range(ktiles):
                            sub = kt % 2
                            if sub == 0:
                                et2 = etp.tile([128, 2048], f8, tag="et",
                                               name="et")
                            base = sub * 1024
                            st2 = stp.tile([128, 1024], f32, tag="st2",
                                           name="st2")
                            j = kt - 4 * qc
                            q0 = max(j, 0) * 128
                            nc.tensor.matmul(
                                st2[:, q0:512],
                                kt_all[0:64, hp * S + kt * 128:
                                       hp * S + (kt + 1) * 128],
                                qt_all[0:64, hp * S + qc * 512 + q0:
                                       hp * S + (qc + 1) * 512],
                                start=True, stop=True, tile_position=(0, 0))
                            nc.tensor.matmul(
                                st2[:, 512 + q0:1024],
                                kt_all[64:128, hp * S + kt * 128:
                                       hp * S + (kt + 1) * 128],
                                qt_all[64:128, hp * S + qc * 512 + q0:
                                       hp * S + (qc + 1) * 512],
                                start=True, stop=True, tile_position=(64, 0))
                            if j > 0:
                                if sub == 1:
                                    nc.gpsimd.memset(
                                        bass_ap_3d(et2, base + (j - 1) * 128,
                                                   512, 2, 128), 0.0)
                                nc.scalar.activation(
                                    bass_ap_3d(et2, base + j * 128, 512, 2,
                                               512 - j * 128),
                                    bass_ap_3d(st2, j * 128, 512, 2,
                                               512 - j * 128),
                                    AF.Exp, scale=0.125)
                            else:
                                nc.scalar.activation(
                                    bass_ap_3d(et2, base, 512, 2, 512),
                                    st2[:], AF.Exp, scale=0.125)
                            if j >= 0:
                                for half in range(2):
                                    blk = et2[:, base + half * 512 + j * 128:
                                              base + half * 512 + (j + 1) * 128]
                                    nc.gpsimd.affine_select(
                                        out=blk, in_=blk, compare_op=ALU.is_ge,
                                        fill=0.0, base=0, pattern=[[1, 128]],
                                        channel_multiplier=-1)
                            if sub == 1:
                                pending.append((kt // 2, et2))
                            if kt == 2:
                                if carry[0] is not None:
                                    carry[0]()
                                    carry[0] = None
                                if posts:
                                    posts.pop(0)()
                            elif kt % 2 == 1 and filler:
                                filler.pop(0)()
                            elif kt == 4 and posts:
                                posts.pop(0)()
                            if sub == 1:
                                av_flush(3)
                        av_flush(2)
                        carry[0] = make_drain(
                            qc, hp, zt0, zt1, pending, npairs,
                            final=(qc == NQ - 1 and hp == 3))

                def tail_units(qc, last=False):
                    """Out-projection of chunk qc as 4 deferred units."""
                    zsr = zrp.tile([128, 512], f16, tag="zsr",
                                   name=f"zsr{qc}")
                    state = {"prepped": False}

                    def prep():
                        # both zsr halves via partition-shifting DMAs on the
                        # idle sync queue (no cast: zsum is already f16)
                        zc = zsum[1:DH + 1, qc * 512:(qc + 1) * 512]
                        nc.sync.dma_start(zsr[0:DH, :], zc)
                        nc.sync.dma_start(zsr[DH:2 * DH, :], zc)

                    def unit(qp):
                        if not state["prepped"]:
                            prep()
                            state["prepped"] = True
                        for sub2 in range(2):
                            for nn in range(2):
                                po = ppp.tile([128, 512], f32, tag="pp",
                                              name="po")
                                if sub2 == 0:
                                    nc.tensor.matmul(
                                        po[:],
                                        zsr[0:DH,
                                            (2 * qp) * 128:(2 * qp + 1) * 128],
                                        wo_sb[0:DH, nn * 512:(nn + 1) * 512],
                                        start=True, stop=True,
                                        tile_position=(0, 0))
                                else:
                                    nc.tensor.matmul(
                                        po[:],
                                        zsr[DH:128, (2 * qp + 1) * 128:
                                            (2 * qp + 2) * 128],
                                        wo_sb[DH:128, nn * 512:(nn + 1) * 512],
                                        start=True, stop=True,
                                        tile_position=(64, 0))
                                osb = osbp.tile([128, 512], f16, tag="osb")
                                if last:
                                    # scalar engine is exp-free by now
                                    nc.scalar.activation(osb[:], po[:],
                                                         AF.Copy)
                                else:
                                    nc.vector.tensor_copy(osb[:], po[:])
                                r0 = qc * 512 + (2 * qp) * 128 + sub2 * 128
                                oeng = nc.sync if last else nc.gpsimd
                                oeng.dma_start(
                                    out[r0:r0 + 128,
                                        nn * 512:(nn + 1) * 512],
                                    osb[:])
                    return [lambda qp=qp: unit(qp) for qp in range(2)]

                # ---------- main schedule ----------
                carry = [None]
                filler = []
                posts = []
                for u in proj_units(0):
                    u()
                for qc in range(NQ):
                    if qc + 1 < NQ:
                        filler.extend(proj_units(qc + 1))
                    attention(qc, carry, filler, posts)
                    # drain leftover proj filler between chunks
                    for u in filler:
                        u()
                    filler = []
                    if qc > 0:
                        # any tail units of chunk qc-1 not yet consumed
                        for u in posts:
                            u()
                        posts = []
                    posts.extend(tail_units(qc, last=(qc == NQ - 1)))
                # final chunk: drain + its tail immediately
                carry[0]()
                carry[0] = None
                for u in posts:
                    u()
    nc.compile()
    return nc


def kernel(**inputs):
    x = np.asarray(inputs["x"], dtype=np.float32)
    WQ = np.asarray(inputs["WQ"], dtype=np.float32)
    bQ = np.asarray(inputs["bQ"], dtype=np.float32)
    WK = np.asarray(inputs["WK"], dtype=np.float32)
    WV = np.asarray(inputs["WV"], dtype=np.float32)
    bV = np.asarray(inputs["bV"], dtype=np.float32)
    WO = np.asarray(inputs["WO"], dtype=np.float32)
    bO = np.asarray(inputs["bO"], dtype=np.float32)

    from concourse.bass_utils import run_bass_kernel_spmd

    if "nc" not in _prog:
        _prog["nc"] = _build()
    nc = _prog["nc"]

    idt_np = np.eye(128, dtype=np.float16)

    def tile_rows(a):
        # [R, C] -> [128, (R//128)*C] with row-block k at cols k*C
        r, c = a.shape
        return np.ascontiguousarray(
            a.reshape(r // 128, 128, c).transpose(1, 0, 2).reshape(128, -1)
        )

    in_maps = []
    for c in range(NCORES):
        b, g = c // 2, c % 2
        sl = slice(g * GD, (g + 1) * GD)
        in_maps.append({
            "x": tile_rows(x[b].astype(np.float16)),
            "wq": tile_rows(WQ[:, sl].astype(np.float16)),
            "wk": tile_rows(WK[:, sl].astype(np.float16)),
            "wv": tile_rows(WV[:, sl].astype(np.float16)),
            "bq": np.ascontiguousarray(bQ[sl]).reshape(1, GD).astype(np.float16),
            "wo": WO.astype(np.float16),
            "idt": idt_np,
        })
    _prog["in_maps"] = in_maps
    globals()["_last_in_maps"] = in_maps
    res = run_bass_kernel_spmd(nc, in_maps, core_ids=list(range(NCORES)))
    _prog["res"] = res
    parts = [r["out"].astype(np.float32) for r in res.results]

    extra = bV.reshape(H, DH).sum(0) @ WO + np.float32(H) * bO
    out = np.empty((B, S, D), dtype=np.float32)
    for b in range(B):
        out[b] = parts[2 * b] + parts[2 * b + 1] + extra
    return out


# revision 44
# speedup vs baseline: 1.0044x; 1.0044x over previous
"""Causal self-attention kernel for 8 Trainium2 NeuronCores.

Sharding: core c -> (batch b = c//2, head-group g = c%2). Each core computes
the attention output contribution of 8 heads for one batch element:
    P_c = (sum_{h in group} softmax(Q_h K_h^T / 8 + causal) V_h) @ WO
Host epilogue: out[b] = P_{2b} + P_{2b+1} + (sum_h bV_h) @ WO + 16*bO
(the V-bias commutes through softmax normalization: softmax rows sum to 1;
the K-bias cancels entirely: softmax((Q+bq)(K+bk)^T) = softmax((Q+bq)K^T)
because Q.bk is constant along the key axis.)

v4 design notes (evolved from v3, 339.7us):
  - Projections and scores fp16; ET fp8e4m3; V as fp8 V8 + fp8 residual R8
    consumed by DoubleRow A@V (unchanged math from v3).
  - K projection carries NO bias -> evicted by ScalarE (activation Copy);
    V8 cast f32->fp8 also on ScalarE; both run in proj phases where the
    scalar engine is otherwise idle. DVE keeps Q bias add + R8 residual.
  - Normalization fused into PSUM eviction: copy the l-row to SBUF, PE-
    broadcast it with a ones[1,64] stationary matmul into a PSUM tile,
    reciprocal_approx_fast, then tensor_tensor multiply zt (PSUM) straight
    into zsum.  The v3 ztall intermediate, its 32 DVE copies, and the
    DRAM-bounce partition broadcast are gone.
  - Cross-head-pair software pipelining: the last AV pairs + the norm chain
    of head-pair hp issue as a deferred "carry" inside hp+1's score stream
    (after 3 score tiles), removing the per-hp PE drain bubble.
  - Projection work is sliced into ~1us filler units; attention(qc)
    consumes units of chunk qc+1's projection at hp boundaries, the rest
    issue between chunks. Keeps the PE fed where ScalarE exp lags.
  - x loaded as column-half whole-chunk DMAs split across the sync and
    vector queues, all issued upfront (xs pool holds all 4 chunks).
  - Output is f16 (host upcasts): halves the out DMA; out DMAs ride the
    gpsimd queue which is idle after the weight loads.
"""
import numpy as np

B, S, D, H, DH = 4, 2048, 1024, 16, 64
HPC = 8            # heads per core
GD = HPC * DH      # 512 = group width
NCORES = 8
NQ = S // 512      # 4 q/s chunks of 512
NKT = S // 128     # 16 k-tiles
NDT = D // 128     # 8 d-tiles

_prog = {}


def bass_ap_3d(tile_t, offset, stride, n, inner):
    """AP view [128p, n, inner] over a tile's free dim: col = offset + i*stride + c."""
    import concourse.bass as bass
    ap = tile_t[:]
    return bass.AP(ap.tensor, ap.offset + offset,
                   [ap.ap[0], [stride, n], [1, inner]])


def _build():
    import concourse.bacc as bacc
    import concourse.tile as tile
    from concourse import mybir
    import concourse.bass as bass

    f32 = mybir.dt.float32
    f16 = mybir.dt.float16
    f8 = mybir.dt.float8e4
    AF = mybir.ActivationFunctionType
    ALU = mybir.AluOpType
    DR = mybir.MatmulPerfMode.DoubleRow

    nc = bacc.Bacc(None, target_bir_lowering=False, debug=False)
    # x arrives host-side TRANSPOSED and pretiled chunk-major:
    #   x[p, c*4096 + j*512 + s] = x_orig[c*512 + s, j*128 + p]
    # so no on-device transposes are needed; weights are pretiled too:
    #   w[p, k*GD + col] = W_orig[k*128 + p, col]
    x = nc.dram_tensor("x", [128, NQ * 4096], f16, kind="ExternalInput")
    wq = nc.dram_tensor("wq", [128, NDT * GD], f16, kind="ExternalInput")
    wk = nc.dram_tensor("wk", [128, NDT * GD], f16, kind="ExternalInput")
    wv = nc.dram_tensor("wv", [128, NDT * GD], f16, kind="ExternalInput")
    bq = nc.dram_tensor("bq", [1, GD], f16, kind="ExternalInput")
    wo = nc.dram_tensor("wo", [DH, D], f16, kind="ExternalInput")
    out = nc.dram_tensor("out", [S, D], f16, kind="ExternalOutput")

    with tile.TileContext(nc) as tc:
        with tc.tile_pool(name="const", bufs=1) as constp, \
             tc.tile_pool(name="big", bufs=1) as bigp:
            # ---- persistent tensors ----
            # chunk c at c*4096, d-tile j at c*4096 + j*512
            xt_all = bigp.tile([128, NQ * 4096], f16, tag="xt")
            qt_all = bigp.tile([128, 4 * S], f16, tag="qt")       # m-tile m at m*S
            kt_all = bigp.tile([128, 4 * S], f16, tag="kt")
            vt_all = bigp.tile([128, NKT * 528], f8, tag="vt")    # ones+V8 cols
            rt_all = bigp.tile([128, NKT * 528], f8, tag="rt")    # fp8 residual
            # rows 1-64 hold sum_h Z_h/l_h (row 0 unused: keeps partition
            # alignment with zt, whose row 0 is the l accumulator); f16 so
            # the tail zsr DMAs don't cast (casting DMAs are gpsimd-only)
            zsum = bigp.tile([DH + 1, S], f16, tag="zsum")

            bq_t = constp.tile([128, 4], f32, tag="bq_t")
            wo_sb = constp.tile([128, D], f16, tag="wo_sb")
            ones_sb = constp.tile([1, DH + 1], f32, tag="ones_sb")

            # ---- input DMAs: x chunk 0 split into column halves across the
            # sync and scalar queues so the first transposes start ~1.5us in;
            # chunks 1-3 follow the weights on the gpsimd queue (needed much
            # later, keeps HBM clear for the weights) ----
            def x_chunk_dma(c, half, eng):
                dst = bass.AP(xt_all[:].tensor,
                              xt_all[:].offset + c * 4096 + half * 2048,
                              [xt_all[:].ap[0], [1, 2048]])
                srcap = bass.AP(x, c * 4096 + half * 2048,
                                [[NQ * 4096, 128], [1, 2048]])
                eng.dma_start(dst, srcap)
            # chunk 0 split across the sync and scalar queues; chunks 1-3
            # issue later (behind the weights) so they don't steal HBM
            # bandwidth from wq/wk/wv
            x_chunk_dma(0, 0, nc.sync)
            x_chunk_dma(0, 1, nc.scalar)
            # bq_t is a casting DMA (f16->f32): gpsimd-only
            nc.gpsimd.dma_start(bq_t[:], bass.AP(bq, 0, [[1, 128], [128, 4]]))

            with tc.tile_pool(name="wts", bufs=1) as wtp, \
                 tc.tile_pool(name="et", bufs=6) as etp, \
                 tc.tile_pool(name="lrow", bufs=2) as lrp, \
                 tc.tile_pool(name="rld", bufs=3, space="DRAM") as rldp, \
                 tc.tile_pool(name="lbi", bufs=2) as lbip, \
                 tc.tile_pool(name="zn", bufs=2) as znp, \
                 tc.tile_pool(name="zr", bufs=2) as zrp, \
                 tc.tile_pool(name="osb", bufs=3) as osbp, \
                 tc.tile_pool(name="stp", bufs=2, space="PSUM") as stp, \
                 tc.tile_pool(name="ppp", bufs=2, space="PSUM") as ppp, \
                 tc.tile_pool(name="ztp", bufs=2, space="PSUM") as ztp:
                wq_all = wtp.tile([128, NDT * GD], f16, tag="wq_all")
                wk_all = wtp.tile([128, NDT * GD], f16, tag="wk_all")
                wv_all = wtp.tile([128, NDT * GD], f16, tag="wv_all")
                for (w_all, w_dram) in ((wq_all, wq), (wk_all, wk),
                                        (wv_all, wv)):
                    nc.gpsimd.dma_start(w_all[:], w_dram[:])
                x_chunk_dma(1, 0, nc.gpsimd)
                x_chunk_dma(1, 1, nc.gpsimd)
                # vt/rt layout per (kt, head): col 0 = ones (the l
                # accumulator lands at PSUM partition 0 so the reciprocal
                # can read it directly), cols 1-64 = V8, col 65 = pad.
                nc.gpsimd.memset(ones_sb[:], 1.0)
                nc.gpsimd.memset(
                    bass_ap_3d(vt_all, 0, 66, NKT * HPC, 1), 1.0)
                nc.gpsimd.memset(
                    bass_ap_3d(vt_all, 65, 66, NKT * HPC, 1), 0.0)
                nc.gpsimd.memset(
                    bass.AP(rt_all[:].tensor, rt_all[:].offset,
                            [rt_all[:].ap[0], [66, NKT * HPC], [65, 2]]),
                    0.0)
                nc.gpsimd.dma_start(wo_sb[0:DH, :], wo[:])
                nc.gpsimd.dma_start(wo_sb[DH:2 * DH, :], wo[:])
                for c in range(2, NQ):
                    x_chunk_dma(c, 0, nc.gpsimd)
                    x_chunk_dma(c, 1, nc.gpsimd)

                # ---------- projection filler units ----------
                def qkproj_unit(c, which, m):
                    # one m-tile (2 heads) of the Q or K projection of chunk c
                    w_all = wq_all if which == 0 else wk_all
                    dest = qt_all if which == 0 else kt_all
                    ps = ppp.tile([128, 512], f32, tag="pp", name="ps")
                    for k in range(NDT):
                        nc.tensor.matmul(
                            ps[:],
                            w_all[:, k * GD + m * 128: k * GD + (m + 1) * 128],
                            xt_all[:, c * 4096 + k * 512:
                                   c * 4096 + (k + 1) * 512],
                            start=(k == 0), stop=(k == NDT - 1))
                    dcols = dest[:, m * S + c * 512: m * S + (c + 1) * 512]
                    if which == 0:
                        nc.vector.tensor_scalar_add(dcols, ps[:],
                                                    bq_t[:, m:m + 1])
                    else:
                        nc.vector.tensor_copy(dcols, ps[:])

                def vproj_unit(c, st4):
                    st = c * 4 + st4
                    ps = ppp.tile([128, 512], f32, tag="pp", name="ps")
                    for k in range(NDT):
                        col = c * 4096 + k * 512 + st4 * 128
                        nc.tensor.matmul(
                            ps[:],
                            xt_all[:, col:col + 128],
                            wv_all[:, k * GD:(k + 1) * GD],
                            start=(k == 0), stop=(k == NDT - 1))
                    dst = bass_ap_3d(vt_all, st * 528 + 1, 66, HPC, DH)
                    srcap = bass_ap_3d(ps, 0, DH, HPC, DH)
                    nc.vector.tensor_copy(dst, srcap)
                    rdst = bass_ap_3d(rt_all, st * 528 + 1, 66, HPC, DH)
                    nc.vector.tensor_tensor(rdst, srcap, dst, op=ALU.subtract)

                def proj_units(c):
                    units = []
                    for which in range(2):
                        for m in range(4):
                            units.append(lambda c=c, w=which, m=m:
                                         qkproj_unit(c, w, m))
                    for st4 in range(4):
                        units.append(lambda c=c, s=st4: vproj_unit(c, s))
                    return units

                # ---------- attention ----------
                def make_drain(qc, hp, zt0, zt1, pending, npairs,
                               final=False):
                    """Deferred: last AVs of (qc,hp), then fused norm."""
                    def av(pp, pet):
                        jz = max(2 * pp - 4 * qc, 0)
                        q0 = jz * 128
                        for half, zt in ((0, zt0), (1, zt1)):
                            eap = bass_ap_3d(pet, half * 512 + q0,
                                             1024, 2, 512 - q0)
                            voff = (2 * pp) * 528 + (2 * hp + half) * 66
                            nc.tensor.matmul(
                                zt[:, q0:512],
                                bass_ap_3d(vt_all, voff, 528, 2, 66),
                                eap, start=(pp == 0), stop=False,
                                perf_mode=DR)
                            nc.tensor.matmul(
                                zt[:, q0:512],
                                bass_ap_3d(rt_all, voff, 528, 2, 66),
                                eap, start=False,
                                stop=(pp == npairs - 1),
                                perf_mode=DR)

                    def drain():
                        while pending:
                            av(*pending.pop(0))
                        # fused normalization: 1/l straight off PSUM row 0,
                        # gpsimd broadcast, multiply into zsum rows 1-64
                        zcols = slice(qc * 512, (qc + 1) * 512)
                        for half, zt in ((0, zt0), (1, zt1)):
                            lrcp = lrp.tile([1, 512], f32, tag="lrcp")
                            nc.vector.reciprocal_approx_fast(
                                out=lrcp[:], in_=zt[0:1, :])
                            lbi = lbip.tile([DH + 1, 512], f32, tag="lbi")
                            if final:
                                # sync is busy with the tail DMAs at the
                                # end; broadcast on the idle PE + scalar
                                lbp = ppp.tile([DH + 1, 512], f32, tag="pp",
                                               name="lbp")
                                nc.tensor.matmul(lbp[:], ones_sb[:],
                                                 lrcp[:], start=True,
                                                 stop=True)
                                nc.scalar.activation(lbi[:], lbp[:],
                                                     AF.Copy)
                            else:
                                # broadcast via DRAM bounce on the idle
                                # sync queue (keeps gpsimd/PE/scalar out
                                # of the DVE-ordered norm chain)
                                rld = rldp.tile([1, 512], f32, tag="rld")
                                nc.sync.dma_start(rld[:], lrcp[:])
                                rap = rld[:]
                                nc.sync.dma_start(
                                    lbi[:],
                                    bass.AP(rap.tensor, rap.offset,
                                            [[0, DH + 1], [1, 512]]))
                            # DVE PSUM reads must sit at partition base 0:
                            # span rows 0-64 (row 0 computes l*(1/l) into
                            # the unused zsum row 0)
                            if 2 * hp + half == 0:
                                nc.vector.tensor_tensor(
                                    zsum[0:DH + 1, zcols], zt[0:DH + 1, :],
                                    lbi[0:DH + 1, :], op=ALU.mult)
                            else:
                                zn = znp.tile([DH + 1, 512], f16, tag="zn")
                                nc.vector.tensor_tensor(
                                    zn[0:DH + 1, :], zt[0:DH + 1, :],
                                    lbi[0:DH + 1, :], op=ALU.mult)
                                nc.vector.tensor_tensor(
                                    zsum[0:DH + 1, zcols],
                                    zsum[0:DH + 1, zcols],
                                    zn[0:DH + 1, :], op=ALU.add)
                    return drain

                def attention(qc, carry, filler, posts):
                    """carry: deferred drain from the previous (qc,hp);
                    filler: proj units to interleave; posts: deferred
                    tail-projection units of the previous chunk."""
                    ktiles = 4 * qc + 4
                    npairs = ktiles // 2
                    for hp in range(4):
                        zt0 = ztp.tile([66, 512], f32, tag="zt", name="zt0")
                        zt1 = ztp.tile([66, 512], f32, tag="zt", name="zt1")
                        pending = []

                        def av_flush(lag):
                            while len(pending) > lag:
                                pp, pet = pending.pop(0)
                                jz = max(2 * pp - 4 * qc, 0)
                                q0 = jz * 128
                                for half, zt in ((0, zt0), (1, zt1)):
                                    eap = bass_ap_3d(pet, half * 512 + q0,
                                                     1024, 2, 512 - q0)
                                    voff = ((2 * pp) * 528
                                            + (2 * hp + half) * 66)
                                    nc.tensor.matmul(
                                        zt[:, q0:512],
                                        bass_ap_3d(vt_all, voff, 528, 2, 66),
                                        eap, start=(pp == 0), stop=False,
                                        perf_mode=DR)
                                    nc.tensor.matmul(
                                        zt[:, q0:512],
                                        bass_ap_3d(rt_all, voff, 528, 2, 66),
                                        eap, start=False,
                                        stop=(pp == npairs - 1),
                                        perf_mode=DR)

                        et2 = None
                        for kt in range(ktiles):
                            sub = kt % 2
                            if sub == 0:
                                et2 = etp.tile([128, 2048], f8, tag="et",
                                               name="et")
                            base = sub * 1024
                            st2 = stp.tile([128, 1024], f32, tag="st2",
                                           name="st2")
                            j = kt - 4 * qc
                            q0 = max(j, 0) * 128
                            nc.tensor.matmul(
                                st2[:, q0:512],
                                kt_all[0:64, hp * S + kt * 128:
                                       hp * S + (kt + 1) * 128],
                                qt_all[0:64, hp * S + qc * 512 + q0:
                                       hp * S + (qc + 1) * 512],
                                start=True, stop=True, tile_position=(0, 0))
                            nc.tensor.matmul(
                                st2[:, 512 + q0:1024],
                                kt_all[64:128, hp * S + kt * 128:
                                       hp * S + (kt + 1) * 128],
                                qt_all[64:128, hp * S + qc * 512 + q0:
                                       hp * S + (qc + 1) * 512],
                                start=True, stop=True, tile_position=(64, 0))
                            if j > 0:
                                if sub == 1:
                                    nc.gpsimd.memset(
                                        bass_ap_3d(et2, base + (j - 1) * 128,
                                                   512, 2, 128), 0.0)
                                nc.scalar.activation(
                                    bass_ap_3d(et2, base + j * 128, 512, 2,
                                               512 - j * 128),
                                    bass_ap_3d(st2, j * 128, 512, 2,
                                               512 - j * 128),
                                    AF.Exp, scale=0.125)
                            else:
                                nc.scalar.activation(
                                    bass_ap_3d(et2, base, 512, 2, 512),
                                    st2[:], AF.Exp, scale=0.125)
                            if j >= 0:
                                for half in range(2):
                                    blk = et2[:, base + half * 512 + j * 128:
                                              base + half * 512 + (j + 1) * 128]
                                    nc.gpsimd.affine_select(
                                        out=blk, in_=blk, compare_op=ALU.is_ge,
                                        fill=0.0, base=0, pattern=[[1, 128]],
                                        channel_multiplier=-1)
                            if sub == 1:
                                pending.append((kt // 2, et2))
                            if kt == 2:
                                if carry[0] is not None:
                                    carry[0]()
                                    carry[0] = None
                                if posts:
                                    posts.pop(0)()
                            elif kt % 2 == 1 and filler:
                                filler.pop(0)()
                            elif kt == 4 and posts:
                                posts.pop(0)()
                            if sub == 1:
                                av_flush(3)
                        av_flush(2)
                        carry[0] = make_drain(
                            qc, hp, zt0, zt1, pending, npairs,
                            final=(qc == NQ - 1 and hp == 3))

                def tail_units(qc, last=False):
                    """Out-projection of chunk qc as 4 deferred units."""
                    zsr = zrp.tile([128, 512], f16, tag="zsr",
                                   name=f"zsr{qc}")
                    state = {"prepped": False}

                    def prep():
                        # both zsr halves via partition-shifting DMAs on the
                        # idle sync queue (no cast: zsum is already f16)
                        zc = zsum[1:DH + 1, qc * 512:(qc + 1) * 512]
                        nc.sync.dma_start(zsr[0:DH, :], zc)
                        nc.sync.dma_start(zsr[DH:2 * DH, :], zc)

                    def unit(qp):
                        if not state["prepped"]:
                            prep()
                            state["prepped"] = True
                        for sub2 in range(2):
                            for nn in range(2):
                                po = ppp.tile([128, 512], f32, tag="pp",
                                              name="po")
                                if sub2 == 0:
                                    nc.tensor.matmul(
                                        po[:],
                                        zsr[0:DH,
                                            (2 * qp) * 128:(2 * qp + 1) * 128],
                                        wo_sb[0:DH, nn * 512:(nn + 1) * 512],
                                        start=True, stop=True,
                                        tile_position=(0, 0))
                                else:
                                    nc.tensor.matmul(
                                        po[:],
                                        zsr[DH:128, (2 * qp + 1) * 128:
                                            (2 * qp + 2) * 128],
                                        wo_sb[DH:128, nn * 512:(nn + 1) * 512],
                                        start=True, stop=True,
                                        tile_position=(64, 0))
                                osb = osbp.tile([128, 512], f16, tag="osb")
                                if last:
                                    # scalar engine is exp-free by now
                                    nc.scalar.activation(osb[:], po[:],
                                                         AF.Copy)
                                else:
                                    nc.vector.tensor_copy(osb[:], po[:])
                                r0 = qc * 512 + (2 * qp) * 128 + sub2 * 128
                                oeng = nc.sync if last else nc.gpsimd
                                oeng.dma_start(
                                    out[r0:r0 + 128,
                                        nn * 512:(nn + 1) * 512],
                                    osb[:])
                    return [lambda qp=qp: unit(qp) for qp in range(2)]

                # ---------- main schedule ----------
                carry = [None]
                filler = []
                posts = []
                for u in proj_units(0):
                    u()
                for qc in range(NQ):
                    if qc + 1 < NQ:
                        filler.extend(proj_units(qc + 1))
                    attention(qc, carry, filler, posts)
                    # drain leftover proj filler between chunks
                    for u in filler:
                        u()
                    filler = []
                    if qc > 0:
                        # any tail units of chunk qc-1 not yet consumed
                        for u in posts:
                            u()
                        posts = []
                    posts.extend(tail_units(qc, last=(qc == NQ - 1)))
                # final chunk: drain + its tail immediately
                carry[0]()
                carry[0] = None
                for u in posts:
                    u()
    nc.compile()
    return nc


def kernel(**inputs):
    x = np.asarray(inputs["x"], dtype=np.float32)
    WQ = np.asarray(inputs["WQ"], dtype=np.float32)
    bQ = np.asarray(inputs["bQ"], dtype=np.float32)
    WK = np.asarray(inputs["WK"], dtype=np.float32)
    WV = np.asarray(inputs["WV"], dtype=np.float32)
    bV = np.asarray(inputs["bV"], dtype=np.float32)
    WO = np.asarray(inputs["WO"], dtype=np.float32)
    bO = np.asarray(inputs["bO"], dtype=np.float32)

    from concourse.bass_utils import run_bass_kernel_spmd

    if "nc" not in _prog:
        _prog["nc"] = _build()
    nc = _prog["nc"]

    def tile_rows(a):
        # [R, C] -> [128, (R//128)*C] with row-block k at cols k*C
        r, c = a.shape
        return np.ascontiguousarray(
            a.reshape(r // 128, 128, c).transpose(1, 0, 2).reshape(128, -1)
        )

    def tile_xt(xb):
        # x [S, D] -> transposed+pretiled [128, NQ*4096]:
        # out[p, c*4096 + j*512 + s] = x[c*512 + s, j*128 + p]
        xt = xb.T.reshape(NDT, 128, NQ, 512)          # [j, p, c, s]
        return np.ascontiguousarray(
            xt.transpose(1, 2, 0, 3).reshape(128, NQ * 4096))

    in_maps = []
    for c in range(NCORES):
        b, g = c // 2, c % 2
        sl = slice(g * GD, (g + 1) * GD)
        in_maps.append({
            "x": tile_xt(x[b].astype(np.float16)),
            "wq": tile_rows(WQ[:, sl].astype(np.float16)),
            "wk": tile_rows(WK[:, sl].astype(np.float16)),
            "wv": tile_rows(WV[:, sl].astype(np.float16)),
            "bq": np.ascontiguousarray(bQ[sl]).reshape(1, GD).astype(np.float16),
            "wo": WO.astype(np.float16),
        })
    _prog["in_maps"] = in_maps
    globals()["_last_in_maps"] = in_maps
    res = run_bass_kernel_spmd(nc, in_maps, core_ids=list(range(NCORES)))
    _prog["res"] = res
    parts = [r["out"].astype(np.float32) for r in res.results]

    extra = bV.reshape(H, DH).sum(0) @ WO + np.float32(H) * bO
    out = np.empty((B, S, D), dtype=np.float32)
    for b in range(B):
        out[b] = parts[2 * b] + parts[2 * b + 1] + extra
    return out


# revision 45
# speedup vs baseline: 1.0586x; 1.0540x over previous
"""Causal self-attention kernel for 8 Trainium2 NeuronCores.

Sharding: core c -> (batch b = c//2, head-group g = c%2). Each core computes
the attention output contribution of 8 heads for one batch element:
    P_c = (sum_{h in group} softmax(Q_h K_h^T / 8 + causal) V_h) @ WO
Host epilogue: out[b] = P_{2b} + P_{2b+1} + (sum_h bV_h) @ WO + 16*bO
(the V-bias commutes through softmax normalization: softmax rows sum to 1;
the K-bias cancels entirely: softmax((Q+bq)(K+bk)^T) = softmax((Q+bq)K^T)
because Q.bk is constant along the key axis.)

v4 design notes (evolved from v3, 339.7us):
  - Projections and scores fp16; ET fp8e4m3; V as fp8 V8 + fp8 residual R8
    consumed by DoubleRow A@V (unchanged math from v3).
  - K projection carries NO bias -> evicted by ScalarE (activation Copy);
    V8 cast f32->fp8 also on ScalarE; both run in proj phases where the
    scalar engine is otherwise idle. DVE keeps Q bias add + R8 residual.
  - Normalization fused into PSUM eviction: copy the l-row to SBUF, PE-
    broadcast it with a ones[1,64] stationary matmul into a PSUM tile,
    reciprocal_approx_fast, then tensor_tensor multiply zt (PSUM) straight
    into zsum.  The v3 ztall intermediate, its 32 DVE copies, and the
    DRAM-bounce partition broadcast are gone.
  - Cross-head-pair software pipelining: the last AV pairs + the norm chain
    of head-pair hp issue as a deferred "carry" inside hp+1's score stream
    (after 3 score tiles), removing the per-hp PE drain bubble.
  - Projection work is sliced into ~1us filler units; attention(qc)
    consumes units of chunk qc+1's projection at hp boundaries, the rest
    issue between chunks. Keeps the PE fed where ScalarE exp lags.
  - x loaded as column-half whole-chunk DMAs split across the sync and
    vector queues, all issued upfront (xs pool holds all 4 chunks).
  - Output is f16 (host upcasts): halves the out DMA; out DMAs ride the
    gpsimd queue which is idle after the weight loads.
"""
import numpy as np

B, S, D, H, DH = 4, 2048, 1024, 16, 64
HPC = 8            # heads per core
GD = HPC * DH      # 512 = group width
NCORES = 8
NQ = S // 512      # 4 q/s chunks of 512
NKT = S // 128     # 16 k-tiles
NDT = D // 128     # 8 d-tiles

_prog = {}


def bass_ap_3d(tile_t, offset, stride, n, inner):
    """AP view [128p, n, inner] over a tile's free dim: col = offset + i*stride + c."""
    import concourse.bass as bass
    ap = tile_t[:]
    return bass.AP(ap.tensor, ap.offset + offset,
                   [ap.ap[0], [stride, n], [1, inner]])


def _build():
    import concourse.bacc as bacc
    import concourse.tile as tile
    from concourse import mybir
    import concourse.bass as bass

    f32 = mybir.dt.float32
    f16 = mybir.dt.float16
    f8 = mybir.dt.float8e4
    AF = mybir.ActivationFunctionType
    ALU = mybir.AluOpType
    DR = mybir.MatmulPerfMode.DoubleRow

    nc = bacc.Bacc(None, target_bir_lowering=False, debug=False)
    # x arrives host-side TRANSPOSED and pretiled chunk-major:
    #   x[p, c*4096 + j*512 + s] = x_orig[c*512 + s, j*128 + p]
    # so no on-device transposes are needed; weights are pretiled too:
    #   w[p, k*GD + col] = W_orig[k*128 + p, col]
    x = nc.dram_tensor("x", [128, NQ * 4096], f16, kind="ExternalInput")
    wq = nc.dram_tensor("wq", [128, NDT * GD], f16, kind="ExternalInput")
    wk = nc.dram_tensor("wk", [128, NDT * GD], f16, kind="ExternalInput")
    wv = nc.dram_tensor("wv", [128, NDT * GD], f16, kind="ExternalInput")
    bq = nc.dram_tensor("bq", [1, GD], f16, kind="ExternalInput")
    wo = nc.dram_tensor("wo", [DH, D], f16, kind="ExternalInput")
    out = nc.dram_tensor("out", [S, D], f16, kind="ExternalOutput")

    with tile.TileContext(nc) as tc:
        with tc.tile_pool(name="const", bufs=1) as constp, \
             tc.tile_pool(name="big", bufs=1) as bigp:
            # ---- persistent tensors ----
            # chunk c at c*4096, d-tile j at c*4096 + j*512
            xt_all = bigp.tile([128, NQ * 4096], f16, tag="xt")
            qt_all = bigp.tile([128, 4 * S], f16, tag="qt")       # m-tile m at m*S
            kt_all = bigp.tile([128, 4 * S], f16, tag="kt")
            vt_all = bigp.tile([128, NKT * 528], f8, tag="vt")    # ones+V8 cols
            rt_all = bigp.tile([128, NKT * 528], f8, tag="rt")    # fp8 residual
            # rows 1-64 hold sum_h Z_h/l_h (row 0 unused: keeps partition
            # alignment with zt, whose row 0 is the l accumulator); f16 so
            # the tail zsr DMAs don't cast (casting DMAs are gpsimd-only)
            zsum = bigp.tile([DH + 1, S], f16, tag="zsum")

            bq_t = constp.tile([128, 4], f32, tag="bq_t")
            wo_sb = constp.tile([128, D], f16, tag="wo_sb")
            ones_sb = constp.tile([1, DH + 1], f32, tag="ones_sb")

            # ---- input DMAs: x chunk 0 split into column halves across the
            # sync and scalar queues so the first transposes start ~1.5us in;
            # chunks 1-3 follow the weights on the gpsimd queue (needed much
            # later, keeps HBM clear for the weights) ----
            def x_chunk_dma(c, half, eng):
                dst = bass.AP(xt_all[:].tensor,
                              xt_all[:].offset + c * 4096 + half * 2048,
                              [xt_all[:].ap[0], [1, 2048]])
                srcap = bass.AP(x, c * 4096 + half * 2048,
                                [[NQ * 4096, 128], [1, 2048]])
                eng.dma_start(dst, srcap)
            # chunk 0 split across the sync and scalar queues; chunks 1-3
            # issue later (behind the weights) so they don't steal HBM
            # bandwidth from wq/wk/wv
            x_chunk_dma(0, 0, nc.sync)
            x_chunk_dma(0, 1, nc.scalar)
            # bq_t is a casting DMA (f16->f32): gpsimd-only
            nc.gpsimd.dma_start(bq_t[:], bass.AP(bq, 0, [[1, 128], [128, 4]]))

            with tc.tile_pool(name="wts", bufs=1) as wtp, \
                 tc.tile_pool(name="et", bufs=6) as etp, \
                 tc.tile_pool(name="lrow", bufs=2) as lrp, \
                 tc.tile_pool(name="rld", bufs=3, space="DRAM") as rldp, \
                 tc.tile_pool(name="lbi", bufs=2) as lbip, \
                 tc.tile_pool(name="zn", bufs=2) as znp, \
                 tc.tile_pool(name="zr", bufs=2) as zrp, \
                 tc.tile_pool(name="osb", bufs=3) as osbp, \
                 tc.tile_pool(name="stp", bufs=2, space="PSUM") as stp, \
                 tc.tile_pool(name="ppp", bufs=2, space="PSUM") as ppp, \
                 tc.tile_pool(name="ztp", bufs=2, space="PSUM") as ztp:
                wq_all = wtp.tile([128, NDT * GD], f16, tag="wq_all")
                wk_all = wtp.tile([128, NDT * GD], f16, tag="wk_all")
                wv_all = wtp.tile([128, NDT * GD], f16, tag="wv_all")
                for (w_all, w_dram) in ((wq_all, wq), (wk_all, wk),
                                        (wv_all, wv)):
                    nc.gpsimd.dma_start(w_all[:], w_dram[:])
                x_chunk_dma(1, 0, nc.sync)
                x_chunk_dma(1, 1, nc.scalar)
                # vt/rt layout per (kt, head): col 0 = ones (the l
                # accumulator lands at PSUM partition 0 so the reciprocal
                # can read it directly), cols 1-64 = V8, col 65 = pad.
                nc.gpsimd.memset(ones_sb[:], 1.0)
                nc.gpsimd.memset(
                    bass_ap_3d(vt_all, 0, 66, NKT * HPC, 1), 1.0)
                nc.gpsimd.memset(
                    bass_ap_3d(vt_all, 65, 66, NKT * HPC, 1), 0.0)
                nc.gpsimd.memset(
                    bass.AP(rt_all[:].tensor, rt_all[:].offset,
                            [rt_all[:].ap[0], [66, NKT * HPC], [65, 2]]),
                    0.0)
                nc.gpsimd.dma_start(wo_sb[0:DH, :], wo[:])
                nc.gpsimd.dma_start(wo_sb[DH:2 * DH, :], wo[:])
                for c in range(2, NQ):
                    x_chunk_dma(c, 0, nc.gpsimd)
                    x_chunk_dma(c, 1, nc.gpsimd)

                # ---------- projection filler units ----------
                def qkproj_unit(c, which, m):
                    # one m-tile (2 heads) of the Q or K projection of chunk c
                    w_all = wq_all if which == 0 else wk_all
                    dest = qt_all if which == 0 else kt_all
                    ps = ppp.tile([128, 512], f32, tag="pp", name="ps")
                    for k in range(NDT):
                        nc.tensor.matmul(
                            ps[:],
                            w_all[:, k * GD + m * 128: k * GD + (m + 1) * 128],
                            xt_all[:, c * 4096 + k * 512:
                                   c * 4096 + (k + 1) * 512],
                            start=(k == 0), stop=(k == NDT - 1))
                    dcols = dest[:, m * S + c * 512: m * S + (c + 1) * 512]
                    if which == 0:
                        nc.vector.tensor_scalar_add(dcols, ps[:],
                                                    bq_t[:, m:m + 1])
                    else:
                        nc.vector.tensor_copy(dcols, ps[:])

                def vproj_unit(c, st4):
                    st = c * 4 + st4
                    ps = ppp.tile([128, 512], f32, tag="pp", name="ps")
                    for k in range(NDT):
                        col = c * 4096 + k * 512 + st4 * 128
                        nc.tensor.matmul(
                            ps[:],
                            xt_all[:, col:col + 128],
                            wv_all[:, k * GD:(k + 1) * GD],
                            start=(k == 0), stop=(k == NDT - 1))
                    dst = bass_ap_3d(vt_all, st * 528 + 1, 66, HPC, DH)
                    srcap = bass_ap_3d(ps, 0, DH, HPC, DH)
                    nc.vector.tensor_copy(dst, srcap)
                    rdst = bass_ap_3d(rt_all, st * 528 + 1, 66, HPC, DH)
                    nc.vector.tensor_tensor(rdst, srcap, dst, op=ALU.subtract)

                def proj_units(c):
                    units = []
                    for which in range(2):
                        for m in range(4):
                            units.append(lambda c=c, w=which, m=m:
                                         qkproj_unit(c, w, m))
                    for st4 in range(4):
                        units.append(lambda c=c, s=st4: vproj_unit(c, s))
                    return units

                # ---------- attention ----------
                def make_drain(qc, hp, zt0, zt1, pending, npairs,
                               final=False):
                    """Deferred: last AVs of (qc,hp), then fused norm."""
                    def av(pp, pet):
                        jz = max(2 * pp - 4 * qc, 0)
                        q0 = jz * 128
                        for half, zt in ((0, zt0), (1, zt1)):
                            eap = bass_ap_3d(pet, half * 512 + q0,
                                             1024, 2, 512 - q0)
                            voff = (2 * pp) * 528 + (2 * hp + half) * 66
                            nc.tensor.matmul(
                                zt[:, q0:512],
                                bass_ap_3d(vt_all, voff, 528, 2, 66),
                                eap, start=(pp == 0), stop=False,
                                perf_mode=DR)
                            nc.tensor.matmul(
                                zt[:, q0:512],
                                bass_ap_3d(rt_all, voff, 528, 2, 66),
                                eap, start=False,
                                stop=(pp == npairs - 1),
                                perf_mode=DR)

                    def drain():
                        while pending:
                            av(*pending.pop(0))
                        # fused normalization: 1/l straight off PSUM row 0,
                        # gpsimd broadcast, multiply into zsum rows 1-64
                        zcols = slice(qc * 512, (qc + 1) * 512)
                        for half, zt in ((0, zt0), (1, zt1)):
                            lrcp = lrp.tile([1, 512], f32, tag="lrcp")
                            nc.vector.reciprocal_approx_fast(
                                out=lrcp[:], in_=zt[0:1, :])
                            lbi = lbip.tile([DH + 1, 512], f32, tag="lbi")
                            if final:
                                # sync is busy with the tail DMAs at the
                                # end; broadcast on the idle PE + scalar
                                lbp = ppp.tile([DH + 1, 512], f32, tag="pp",
                                               name="lbp")
                                nc.tensor.matmul(lbp[:], ones_sb[:],
                                                 lrcp[:], start=True,
                                                 stop=True)
                                nc.scalar.activation(lbi[:], lbp[:],
                                                     AF.Copy)
                            else:
                                nc.gpsimd.partition_broadcast(
                                    lbi[:], lrcp[:], channels=DH + 1)
                            # DVE PSUM reads must sit at partition base 0:
                            # span rows 0-64 (row 0 computes l*(1/l) into
                            # the unused zsum row 0)
                            if 2 * hp + half == 0:
                                nc.vector.tensor_tensor(
                                    zsum[0:DH + 1, zcols], zt[0:DH + 1, :],
                                    lbi[0:DH + 1, :], op=ALU.mult)
                            else:
                                zn = znp.tile([DH + 1, 512], f16, tag="zn")
                                nc.vector.tensor_tensor(
                                    zn[0:DH + 1, :], zt[0:DH + 1, :],
                                    lbi[0:DH + 1, :], op=ALU.mult)
                                nc.vector.tensor_tensor(
                                    zsum[0:DH + 1, zcols],
                                    zsum[0:DH + 1, zcols],
                                    zn[0:DH + 1, :], op=ALU.add)
                    return drain

                def attention(qc, carry, filler, posts):
                    """carry: deferred drain from the previous (qc,hp);
                    filler: proj units to interleave; posts: deferred
                    tail-projection units of the previous chunk."""
                    ktiles = 4 * qc + 4
                    npairs = ktiles // 2
                    for hp in range(4):
                        zt0 = ztp.tile([66, 512], f32, tag="zt", name="zt0")
                        zt1 = ztp.tile([66, 512], f32, tag="zt", name="zt1")
                        pending = []

                        def av_flush(lag):
                            while len(pending) > lag:
                                pp, pet = pending.pop(0)
                                jz = max(2 * pp - 4 * qc, 0)
                                q0 = jz * 128
                                for half, zt in ((0, zt0), (1, zt1)):
                                    eap = bass_ap_3d(pet, half * 512 + q0,
                                                     1024, 2, 512 - q0)
                                    voff = ((2 * pp) * 528
                                            + (2 * hp + half) * 66)
                                    nc.tensor.matmul(
                                        zt[:, q0:512],
                                        bass_ap_3d(vt_all, voff, 528, 2, 66),
                                        eap, start=(pp == 0), stop=False,
                                        perf_mode=DR)
                                    nc.tensor.matmul(
                                        zt[:, q0:512],
                                        bass_ap_3d(rt_all, voff, 528, 2, 66),
                                        eap, start=False,
                                        stop=(pp == npairs - 1),
                                        perf_mode=DR)

                        et2 = None
                        for kt in range(ktiles):
                            sub = kt % 2
                            if sub == 0:
                                et2 = etp.tile([128, 2048], f8, tag="et",
                                               name="et")
                            base = sub * 1024
                            st2 = stp.tile([128, 1024], f32, tag="st2",
                                           name="st2")
                            j = kt - 4 * qc
                            q0 = max(j, 0) * 128
                            nc.tensor.matmul(
                                st2[:, q0:512],
                                kt_all[0:64, hp * S + kt * 128:
                                       hp * S + (kt + 1) * 128],
                                qt_all[0:64, hp * S + qc * 512 + q0:
                                       hp * S + (qc + 1) * 512],
                                start=True, stop=True, tile_position=(0, 0))
                            nc.tensor.matmul(
                                st2[:, 512 + q0:1024],
                                kt_all[64:128, hp * S + kt * 128:
                                       hp * S + (kt + 1) * 128],
                                qt_all[64:128, hp * S + qc * 512 + q0:
                                       hp * S + (qc + 1) * 512],
                                start=True, stop=True, tile_position=(64, 0))
                            if j > 0:
                                if sub == 1:
                                    nc.gpsimd.memset(
                                        bass_ap_3d(et2, base + (j - 1) * 128,
                                                   512, 2, 128), 0.0)
                                nc.scalar.activation(
                                    bass_ap_3d(et2, base + j * 128, 512, 2,
                                               512 - j * 128),
                                    bass_ap_3d(st2, j * 128, 512, 2,
                                               512 - j * 128),
                                    AF.Exp, scale=0.125)
                            else:
                                nc.scalar.activation(
                                    bass_ap_3d(et2, base, 512, 2, 512),
                                    st2[:], AF.Exp, scale=0.125)
                            if j >= 0:
                                for half in range(2):
                                    blk = et2[:, base + half * 512 + j * 128:
                                              base + half * 512 + (j + 1) * 128]
                                    nc.gpsimd.affine_select(
                                        out=blk, in_=blk, compare_op=ALU.is_ge,
                                        fill=0.0, base=0, pattern=[[1, 128]],
                                        channel_multiplier=-1)
                            if sub == 1:
                                pending.append((kt // 2, et2))
                            if kt == 2:
                                if carry[0] is not None:
                                    carry[0]()
                                    carry[0] = None
                                if posts:
                                    posts.pop(0)()
                            elif kt % 2 == 1 and filler:
                                filler.pop(0)()
                            elif kt == 4 and posts:
                                posts.pop(0)()
                            if sub == 1:
                                av_flush(3)
                        av_flush(2)
                        carry[0] = make_drain(
                            qc, hp, zt0, zt1, pending, npairs,
                            final=(qc == NQ - 1 and hp == 3))

                def tail_units(qc, last=False):
                    """Out-projection of chunk qc as 4 deferred units."""
                    zsr = zrp.tile([128, 512], f16, tag="zsr",
                                   name=f"zsr{qc}")
                    state = {"prepped": False}

                    def prep():
                        # both zsr halves via partition-shifting DMAs on the
                        # idle sync queue (no cast: zsum is already f16)
                        zc = zsum[1:DH + 1, qc * 512:(qc + 1) * 512]
                        nc.sync.dma_start(zsr[0:DH, :], zc)
                        nc.sync.dma_start(zsr[DH:2 * DH, :], zc)

                    def unit(qp):
                        if not state["prepped"]:
                            prep()
                            state["prepped"] = True
                        for sub2 in range(2):
                            for nn in range(2):
                                po = ppp.tile([128, 512], f32, tag="pp",
                                              name="po")
                                if sub2 == 0:
                                    nc.tensor.matmul(
                                        po[:],
                                        zsr[0:DH,
                                            (2 * qp) * 128:(2 * qp + 1) * 128],
                                        wo_sb[0:DH, nn * 512:(nn + 1) * 512],
                                        start=True, stop=True,
                                        tile_position=(0, 0))
                                else:
                                    nc.tensor.matmul(
                                        po[:],
                                        zsr[DH:128, (2 * qp + 1) * 128:
                                            (2 * qp + 2) * 128],
                                        wo_sb[DH:128, nn * 512:(nn + 1) * 512],
                                        start=True, stop=True,
                                        tile_position=(64, 0))
                                osb = osbp.tile([128, 512], f16, tag="osb")
                                if last:
                                    # scalar engine is exp-free by now
                                    nc.scalar.activation(osb[:], po[:],
                                                         AF.Copy)
                                else:
                                    nc.vector.tensor_copy(osb[:], po[:])
                                r0 = qc * 512 + (2 * qp) * 128 + sub2 * 128
                                oeng = nc.sync if last else nc.gpsimd
                                oeng.dma_start(
                                    out[r0:r0 + 128,
                                        nn * 512:(nn + 1) * 512],
                                    osb[:])
                    return [lambda qp=qp: unit(qp) for qp in range(2)]

                # ---------- main schedule ----------
                carry = [None]
                filler = []
                posts = []
                for u in proj_units(0):
                    u()
                for qc in range(NQ):
                    if qc + 1 < NQ:
                        filler.extend(proj_units(qc + 1))
                    attention(qc, carry, filler, posts)
                    # drain leftover proj filler between chunks
                    for u in filler:
                        u()
                    filler = []
                    if qc > 0:
                        # any tail units of chunk qc-1 not yet consumed
                        for u in posts:
                            u()
                        posts = []
                    posts.extend(tail_units(qc, last=(qc == NQ - 1)))
                # final chunk: drain + its tail immediately
                carry[0]()
                carry[0] = None
                for u in posts:
                    u()
    nc.compile()
    return nc


def kernel(**inputs):
    x = np.asarray(inputs["x"], dtype=np.float32)
    WQ = np.asarray(inputs["WQ"], dtype=np.float32)
    bQ = np.asarray(inputs["bQ"], dtype=np.float32)
    WK = np.asarray(inputs["WK"], dtype=np.float32)
    WV = np.asarray(inputs["WV"], dtype=np.float32)
    bV = np.asarray(inputs["bV"], dtype=np.float32)
    WO = np.asarray(inputs["WO"], dtype=np.float32)
    bO = np.asarray(inputs["bO"], dtype=np.float32)

    from concourse.bass_utils import run_bass_kernel_spmd

    if "nc" not in _prog:
        _prog["nc"] = _build()
    nc = _prog["nc"]

    def tile_rows(a):
        # [R, C] -> [128, (R//128)*C] with row-block k at cols k*C
        r, c = a.shape
        return np.ascontiguousarray(
            a.reshape(r // 128, 128, c).transpose(1, 0, 2).reshape(128, -1)
        )

    def tile_xt(xb):
        # x [S, D] -> transposed+pretiled [128, NQ*4096]:
        # out[p, c*4096 + j*512 + s] = x[c*512 + s, j*128 + p]
        xt = xb.T.reshape(NDT, 128, NQ, 512)          # [j, p, c, s]
        return np.ascontiguousarray(
            xt.transpose(1, 2, 0, 3).reshape(128, NQ * 4096))

    in_maps = []
    for c in range(NCORES):
        b, g = c // 2, c % 2
        sl = slice(g * GD, (g + 1) * GD)
        in_maps.append({
            "x": tile_xt(x[b].astype(np.float16)),
            "wq": tile_rows(WQ[:, sl].astype(np.float16)),
            "wk": tile_rows(WK[:, sl].astype(np.float16)),
            "wv": tile_rows(WV[:, sl].astype(np.float16)),
            "bq": np.ascontiguousarray(bQ[sl]).reshape(1, GD).astype(np.float16),
            "wo": WO.astype(np.float16),
        })
    _prog["in_maps"] = in_maps
    globals()["_last_in_maps"] = in_maps
    res = run_bass_kernel_spmd(nc, in_maps, core_ids=list(range(NCORES)))
    _prog["res"] = res
    parts = [r["out"].astype(np.float32) for r in res.results]

    extra = bV.reshape(H, DH).sum(0) @ WO + np.float32(H) * bO
    out = np.empty((B, S, D), dtype=np.float32)
    for b in range(B):
        out[b] = parts[2 * b] + parts[2 * b + 1] + extra
    return out
